# revision 39
# baseline (speedup 1.0000x reference)
"""HGT (heterogeneous graph transformer) layer on 8 trn2 NeuronCores.

Strategy (dst-node 1D sharding, uniform SPMD program, bf16 compute):
  - Host folds all small weights (bf16):
      WKV[t]    = [W_k[t] | W_v[t]]                     (node-type projections)
      WQA[t,r]  = W_q[t] @ blockdiag(W_att[r]) * pri[r,h]/sqrt(dk)
      WMO[r,t]  = blockdiag(W_msg[r]) @ (sigmoid(skip[t])*W_a[t])
    so the per-edge computation needs only RAW k/v rows of the src node:
      attn[e,h] = qat[rel][dst] . k_raw[src]    (per head, pri pre-folded)
      agg[j]    = sum_r (sum_{e in rel r, dst=j} w_e * v_raw[src]) @ WMO[r]
      out[j]    = agg[j] / s[j]                 (softmax denominator)
  - h is pre-transposed/bf16 on the host ([128 d, N]), so phase 1 (k|v table
    build) is transpose-free: one bf16 matmul per 128-node tile, stored to a
    DRAM table split at node 25600 into lo/hi halves so dma_gather's int16
    row indices stay in range.
  - Each core owns 6400 contiguous dst nodes (50 tiles of 128). Edges are
    grouped host-side into (tile, relation, half) chunks of 128; chunk
    structure is the max over cores so the SPMD program is identical, with
    per-core data padded (pad rows gather table row 0 and carry dst 999 so
    the one-hot mask zeroes their contribution).
  - Phase 2 gathers each 3-tile group's k|v rows with two dma_gather calls
    (lo+hi), then per tile: one-hot O via DVE is_equal, per chunk a bf16
    transpose + one-hot-gather matmul for q_att[dst], dot with k via
    Pool mult + DVE reduce, exp on Act, weighted v via batched DVE mult,
    and per-relation segment sums as PSUM-accumulated matmuls.
  - Softmax skips the segment-max subtraction (scores are O(1) here).
"""

import sys

sys.path.insert(0, "/opt/trn_rl_repo")

import numpy as np
import ml_dtypes

import concourse.bacc as bacc_mod
import concourse.mybir as mybir
import concourse.tile as tile_mod
from concourse import library_config
from concourse.bass_utils import run_bass_kernel_spmd
from concourse.masks import make_identity

F32 = mybir.dt.float32
BF16 = mybir.dt.bfloat16
I16 = mybir.dt.int16

N, E, T, R, NH, DK, D = 51200, 640000, 4, 8, 4, 32, 128
NCORES = 8
NPC = N // NCORES          # 6400 nodes per core
TPC = NPC // 128           # 50 dst tiles per core
TT = N // 128              # 400 table tiles
NPT = N // T               # nodes per type
EPR = E // R               # edges per relation
SPLIT = 25600              # kv table lo/hi split (int16 gather indices)
SPLIT_TILE = SPLIT // 128  # 200
GSZ = 3                    # tiles per gather group
SQRT_DK = float(np.sqrt(DK))

BF = ml_dtypes.bfloat16


def _blockdiag(W):
    out = np.zeros((R, D, D), np.float32)
    for r in range(R):
        for hh in range(NH):
            out[r, hh * DK:(hh + 1) * DK, hh * DK:(hh + 1) * DK] = W[r, hh]
    return out


def _pack16(idx):
    """[n] int16 (n % 128 == 0) -> [128, n//16] wrapped in 16 partitions."""
    n = len(idx)
    a = np.zeros((16, n // 16), np.int16)
    a[np.arange(n) % 16, np.arange(n) // 16] = idx
    return np.tile(a, (8, 1))


def _host_prep(h, k_linears, q_linears, v_linears, a_linears,
               relation_att, relation_msg, relation_pri, skip,
               row_idx, col_idx):
    Watt = _blockdiag(np.asarray(relation_att, np.float32))
    Wmsg = _blockdiag(np.asarray(relation_msg, np.float32))
    skip = np.asarray(skip, np.float32)
    Wout = (1.0 / (1.0 + np.exp(-skip))).astype(np.float32) * np.asarray(a_linears, np.float32)
    WQA = np.einsum("tab,rbc->trac", np.asarray(q_linears, np.float32), Watt)
    # fold relation prior / sqrt(dk) into the q rotation (per head columns)
    pri = np.asarray(relation_pri, np.float32) / SQRT_DK               # [R,H]
    WQA = WQA.reshape(T, R, D, NH, DK) * pri[None, :, None, :, None]
    WQA = WQA.reshape(T, R, D, D)
    WMO = np.einsum("rab,tbc->rtac", Wmsg, Wout)
    WKV = np.concatenate([np.asarray(k_linears, np.float32),
                          np.asarray(v_linears, np.float32)], axis=2)  # [T,D,256]

    h = np.asarray(h, np.float32)
    hT = np.ascontiguousarray(h.T.astype(BF))                          # [128, N]

    row = np.asarray(row_idx, np.int64)
    col = np.asarray(col_idx, np.int64)
    erel = np.arange(E, dtype=np.int64) // EPR
    half = (row >= SPLIT).astype(np.int64)

    core = col // NPC
    tl = (col % NPC) // 128
    # group key: (core, tile, rel, half)
    key = ((core * TPC + tl) * R + erel) * 2 + half
    nkeys = NCORES * TPC * R * 2
    counts = np.bincount(key, minlength=nkeys).reshape(NCORES, TPC, R, 2)
    maxcnt = counts.max(axis=0)                                        # [TPC,R,2]
    n_chunks = -(-maxcnt // 128)                                       # ceil, may be 0

    # chunk layout per tile: all lo chunks (by rel), then all hi chunks
    chunk_rel = []     # [tl] -> list of rel per chunk
    chunk_half = []    # [tl] -> list of half per chunk
    chunk_base = np.zeros((TPC, R, 2), np.int64)   # chunk index within tile
    CLO = np.zeros(TPC, np.int64)
    C_t = np.zeros(TPC, np.int64)
    for t in range(TPC):
        rels, halves = [], []
        off = 0
        for hf in range(2):
            for r in range(R):
                chunk_base[t, r, hf] = off
                nc_ = int(n_chunks[t, r, hf])
                rels += [r] * nc_
                halves += [hf] * nc_
                off += nc_
            if hf == 0:
                CLO[t] = off
        C_t[t] = off
        chunk_rel.append(rels)
        chunk_half.append(halves)

    tile_chunk0 = np.zeros(TPC + 1, np.int64)
    np.cumsum(C_t, out=tile_chunk0[1:])
    total_chunks = int(tile_chunk0[-1])

    # per-edge rank within its (core,tile,rel,half) group
    order = np.argsort(key, kind="stable")
    group_start = np.zeros(nkeys, np.int64)
    cnt_flat = counts.reshape(-1)
    np.cumsum(cnt_flat[:-1], out=group_start[1:])
    ranks = np.empty(E, np.int64)
    ranks[order] = np.arange(E) - group_start[key[order]]

    chunk_of = tile_chunk0[tl] + chunk_base[tl, erel, half] + ranks // 128
    part_of = ranks % 128

    idx_slots = np.zeros((NCORES, total_chunks, 128), np.int16)
    rds_slots = np.full((NCORES, total_chunks, 128), 999.0, np.float32)
    idx_slots[core, chunk_of, part_of] = (row - SPLIT * half).astype(np.int16)
    rds_slots[core, chunk_of, part_of] = (col % 128).astype(np.float32)

    # gather groups of GSZ tiles: lo stream then hi stream per group
    groups = [list(range(g, min(g + GSZ, TPC))) for g in range(0, TPC, GSZ)]
    group_meta = []   # (glo, ghi, lo_off16, hi_off16, kvg_col per tile chunk)
    idx_streams = [[] for _ in range(NCORES)]
    off16 = 0
    # kvg column (chunk slot within group tile) per global chunk
    kvg_col = np.zeros(total_chunks, np.int64)
    for tiles in groups:
        glo = int(sum(CLO[t] for t in tiles))
        ghi = int(sum(C_t[t] - CLO[t] for t in tiles))
        # lo chunks first
        colp = 0
        for t in tiles:
            c0 = tile_chunk0[t]
            for c in range(int(CLO[t])):
                kvg_col[c0 + c] = colp
                colp += 1
        for t in tiles:
            c0 = tile_chunk0[t]
            for c in range(int(CLO[t]), int(C_t[t])):
                kvg_col[c0 + c] = colp
                colp += 1
        lo_off16 = off16
        hi_off16 = off16 + glo * 8
        off16 += (glo + ghi) * 8
        group_meta.append((tiles, glo, ghi, lo_off16, hi_off16))
        for cc in range(NCORES):
            lo_idx = [idx_slots[cc, tile_chunk0[t] + c]
                      for t in tiles for c in range(int(CLO[t]))]
            hi_idx = [idx_slots[cc, tile_chunk0[t] + c]
                      for t in tiles for c in range(int(CLO[t]), int(C_t[t]))]
            idx_streams[cc].append(np.concatenate(lo_idx) if lo_idx else
                                   np.zeros(0, np.int16))
            idx_streams[cc].append(np.concatenate(hi_idx) if hi_idx else
                                   np.zeros(0, np.int16))

    idx16 = [
        _pack16(np.concatenate(idx_streams[cc]))
        for cc in range(NCORES)
    ]
    idx_cols = idx16[0].shape[1]

    meta = dict(
        chunk_rel=chunk_rel, chunk_half=chunk_half, C_t=C_t, CLO=CLO,
        tile_chunk0=tile_chunk0, total_chunks=total_chunks,
        group_meta=group_meta, kvg_col=kvg_col, idx_cols=idx_cols,
    )

    in_maps = []
    for cc in range(NCORES):
        t_c = (cc * NPC) // NPT
        wqa = WQA[t_c].transpose(1, 0, 2).reshape(D, R * D).astype(BF)
        wmo = WMO[:, t_c].transpose(1, 0, 2).reshape(D, R * D).astype(BF)
        wkv = WKV.transpose(1, 0, 2).reshape(D, T * 256).astype(BF)
        in_maps.append({
            "hT": hT,
            "hto": np.ascontiguousarray(hT[:, cc * NPC:(cc + 1) * NPC]),
            "wkv": np.ascontiguousarray(wkv),
            "wqa": np.ascontiguousarray(wqa),
            "wmo": np.ascontiguousarray(wmo),
            "idx16": idx16[cc],
            # precomputed one-hot O[e, (chunk, j)] (exact in bf16): frees
            # the DVE is_equal build; DMA-loaded per tile instead
            "oallh": np.ascontiguousarray(
                (rds_slots[cc][None, :, :] ==
                 np.arange(128, dtype=np.float32)[:, None, None])
                .transpose(2, 1, 0)          # -> [e, chunk, j]
                .reshape(128, total_chunks * 128).astype(BF)),
        })
    return in_maps, meta


def _build_program(meta):
    chunk_rel = meta["chunk_rel"]
    chunk_half = meta["chunk_half"]
    C_t = meta["C_t"]
    CLO = meta["CLO"]
    tile_chunk0 = meta["tile_chunk0"]
    total_chunks = meta["total_chunks"]
    group_meta = meta["group_meta"]
    kvg_col = meta["kvg_col"]
    idx_cols = meta["idx_cols"]

    nc = bacc_mod.Bacc(num_swdge_queues=4)
    hT_ext = nc.declare_dram_parameter("hT", [128, N], BF16, isOutput=False)
    hto_ext = nc.declare_dram_parameter("hto", [128, NPC], BF16, isOutput=False)
    wkv_ext = nc.declare_dram_parameter("wkv", [128, T * 256], BF16, isOutput=False)
    wqa_ext = nc.declare_dram_parameter("wqa", [128, R * D], BF16, isOutput=False)
    wmo_ext = nc.declare_dram_parameter("wmo", [128, R * D], BF16, isOutput=False)
    idx_ext = nc.declare_dram_parameter("idx16", [128, idx_cols], I16, isOutput=False)
    oall_ext = nc.declare_dram_parameter("oallh", [128, total_chunks * 128],
                                         BF16, isOutput=False)
    out_ext = nc.declare_dram_parameter("out", [NPC, D], F32, isOutput=True)
    out_t = out_ext[:].rearrange("(t p) x -> p t x", p=128)

    kvlo = nc.dram_tensor("kvlo", [SPLIT, 256], BF16)
    kvhi = nc.dram_tensor("kvhi", [N - SPLIT, 256], BF16)
    kvlo_t = kvlo[:].rearrange("(t p) x -> p t x", p=128)   # [128, 200, 256]
    kvhi_t = kvhi[:].rearrange("(t p) x -> p t x", p=128)   # [128, 200, 256]

    with tile_mod.TileContext(nc) as tc:
        with tc.tile_pool(name="const", bufs=1) as cp:
            nc.gpsimd.load_library(library_config.mlp)
            ident = cp.tile([128, 128], BF16)
            make_identity(nc, ident[:])
            wkv_sb = cp.tile([128, T * 256], BF16)
            nc.sync.dma_start(out=wkv_sb[:], in_=wkv_ext[:])
            wqa_sb = cp.tile([128, R * D], BF16)
            nc.sync.dma_start(out=wqa_sb[:], in_=wqa_ext[:])
            wmo_sb = cp.tile([128, R * D], BF16)
            nc.sync.dma_start(out=wmo_sb[:], in_=wmo_ext[:])
            hto_sb = cp.tile([128, NPC], BF16)
            nc.sync.dma_start(out=hto_sb[:], in_=hto_ext[:])
            idx_sb = cp.tile([128, idx_cols], I16)
            nc.sync.dma_start(out=idx_sb[:], in_=idx_ext[:])

            # ---- phase 1: k|v table for all N nodes (bf16, transpose-free)
            # Emitted as lo segments (tiles 0..199 -> kvlo) then hi; the
            # first gather groups' lo-half gathers are emitted in between so
            # they overlap the hi half of the table build.
            kvgp_cm = tc.tile_pool(name="kvg", bufs=3)
            kvgp = kvgp_cm.__enter__()
            prefetched = {}
            qnum = 0

            def emit_gathers(kvgt, glo, ghi, lo_off16, hi_off16, want_half):
                nonlocal qnum
                for half, nch, tab, off16 in (
                    (0, glo, kvlo, lo_off16),
                    (1, ghi, kvhi, hi_off16),
                ):
                    if half != want_half:
                        continue
                    base = 0 if half == 0 else glo
                    for ps in range(0, nch, 8):
                        pe_ = min(ps + 8, nch)
                        nc.gpsimd.dma_gather(
                            kvgt[:, (base + ps) * 256:(base + pe_) * 256]
                                .rearrange("p (c x) -> p c x", x=256),
                            tab[:],
                            idx_sb[:, off16 + ps * 8:off16 + pe_ * 8],
                            (pe_ - ps) * 128, (pe_ - ps) * 128, 256,
                            queue_num=qnum % 4)
                        qnum += 1

            with (
                tc.tile_pool(name="hseg", bufs=2) as hsp,
                tc.tile_pool(name="kvs", bufs=6) as kvsp,
                tc.tile_pool(name="ps1", bufs=8, space="PSUM") as ps1,
            ):
                SEG = 6400          # nodes per streamed hT segment
                STILES = SEG // 128

                def phase1_seg(seg):
                    hseg = hsp.tile([128, SEG], BF16, tag="hseg")
                    nc.sync.dma_start(out=hseg[:],
                                      in_=hT_ext[:, seg * SEG:(seg + 1) * SEG])
                    for tp in range(0, STILES, 4):
                        nst = min(4, STILES - tp)
                        kvs = kvsp.tile([128, 4 * 256], BF16, tag="kvs")
                        for u in range(nst):
                            t = seg * STILES + tp + u
                            ty = t // (TT // T)
                            kvp = ps1.tile([128, 256], F32, tag="kvp")
                            nc.tensor.matmul(
                                kvp[:],
                                lhsT=hseg[:, (tp + u) * 128:(tp + u + 1) * 128],
                                rhs=wkv_sb[:, ty * 256:(ty + 1) * 256],
                                start=True, stop=True)
                            if u % 2 == 0:
                                nc.vector.tensor_copy(
                                    kvs[:, u * 256:(u + 1) * 256], kvp[:])
                            else:
                                nc.scalar.copy(
                                    kvs[:, u * 256:(u + 1) * 256], kvp[:])
                        t0 = seg * STILES + tp
                        if t0 < SPLIT_TILE:
                            nc.sync.dma_start(
                                out=kvlo_t[:, t0:t0 + nst, :],
                                in_=kvs[:, :nst * 256]
                                    .rearrange("p (t x) -> p t x", t=nst))
                        else:
                            nc.sync.dma_start(
                                out=kvhi_t[:, t0 - SPLIT_TILE:t0 - SPLIT_TILE + nst, :],
                                in_=kvs[:, :nst * 256]
                                    .rearrange("p (t x) -> p t x", t=nst))

                for seg in range(4):            # lo half (tiles 0..199)
                    phase1_seg(seg)
                for seg in range(4, N // SEG):  # hi half
                    phase1_seg(seg)

            # ---- phase 2: per dst-tile edge processing ----
            # Engine budget: Pool cannot read PSUM, so DVE owns the
            # PSUM-sourced elementwise (prod, denominators); Pool gets the
            # SBUF-only builds (one-hot O, most of wmt) plus SWDGE gather
            # generation; Act does PSUM->SBUF copies (Ots, qat, AT).
            with (
                tc.tile_pool(name="oall", bufs=3) as op_,
                tc.tile_pool(name="prod", bufs=3) as prp,
                tc.tile_pool(name="wmt", bufs=3) as wmp,
                tc.tile_pool(name="qat", bufs=3) as qtp,
                tc.tile_pool(name="ots", bufs=3) as otsp,
                tc.tile_pool(name="ats", bufs=2) as atsp,
                tc.tile_pool(name="sm", bufs=3) as smp,
                tc.tile_pool(name="an", bufs=3) as anp,
                tc.tile_pool(name="qps", bufs=1, space="PSUM") as qps,
                tc.tile_pool(name="otps", bufs=2, space="PSUM") as otps,
                tc.tile_pool(name="qeps", bufs=2, space="PSUM") as qeps,
                tc.tile_pool(name="atps", bufs=1, space="PSUM") as atps,
                tc.tile_pool(name="spps", bufs=1, space="PSUM") as spps,
            ):
                qnum = 0
                for tiles, glo, ghi, lo_off16, hi_off16 in group_meta:
                    gtot = glo + ghi
                    kvgt = kvgp.tile([128, gtot * 256], BF16, tag="kvg")
                    # SWDGE ring caps one gather at 1024 descriptors (8
                    # chunks); emit pieces round-robined over 4 queues so
                    # descriptor generation parallelizes across Q7 pairs.
                    for half, nch, tab, off16 in (
                        (0, glo, kvlo, lo_off16),
                        (1, ghi, kvhi, hi_off16),
                    ):
                        base = 0 if half == 0 else glo
                        for ps in range(0, nch, 8):
                            pe_ = min(ps + 8, nch)
                            npc_ = pe_ - ps
                            nc.gpsimd.dma_gather(
                                kvgt[:, (base + ps) * 256:(base + pe_) * 256]
                                    .rearrange("p (c x) -> p c x", x=256),
                                tab[:],
                                idx_sb[:, off16 + ps * 8:off16 + pe_ * 8],
                                npc_ * 128, npc_ * 128, 256,
                                queue_num=qnum % 4)
                            qnum += 1

                    for tl in tiles:
                        C = int(C_t[tl])
                        clo = int(CLO[tl])
                        c0 = int(tile_chunk0[tl])
                        rels = chunk_rel[tl]
                        # runs of <=4 chunks, contiguous in kvg (cut at spans)
                        bounds = [0, clo, C] if 0 < clo < C else [0, C]
                        runs = []
                        for bi in range(len(bounds) - 1):
                            a, b = bounds[bi], bounds[bi + 1]
                            for s in range(a, b, 4):
                                runs.append((s, min(s + 4, b)))
                        spans = [(bounds[i], bounds[i + 1])
                                 for i in range(len(bounds) - 1)]

                        # q rotated by all relations for this tile's dst rows
                        qat = qtp.tile([128, R * D], BF16, tag="qat")
                        for uu in range(2):
                            qp = qps.tile([128, 512], F32, tag="qp")
                            nc.tensor.matmul(
                                qp[:],
                                lhsT=hto_sb[:, tl * 128:(tl + 1) * 128],
                                rhs=wqa_sb[:, uu * 512:(uu + 1) * 512],
                                start=True, stop=True)
                            nc.scalar.copy(qat[:, uu * 512:(uu + 1) * 512], qp[:])

                        # one-hot O[e, j]: host-precomputed, DMA-loaded
                        # (no upstream dep -> prefetches freely)
                        Oall = op_.tile([128, C * 128], BF16, tag="Oall")
                        nc.sync.dma_start(
                            out=Oall[:],
                            in_=oall_ext[:, c0 * 128:(c0 + C) * 128])

                        # OT per chunk; 8 transposes share one PSUM bank,
                        # one Act copy per bank -> Ots in SBUF
                        n8 = -(-C // 8)
                        ots_sb = otsp.tile([128, n8 * 8 * 128], BF16, tag="ots")
                        for b8 in range(n8):
                            ca, cb = b8 * 8, min(b8 * 8 + 8, C)
                            OTq = otps.tile([128, 8 * 128], BF16, tag="OTq")
                            for c in range(ca, cb):
                                nc.tensor.transpose(
                                    OTq[:, (c - ca) * 128:(c - ca + 1) * 128],
                                    Oall[:, c * 128:(c + 1) * 128], ident[:])
                                if c + 1 == cb or (c - ca) == 3:
                                    nc.scalar.copy(
                                        ots_sb[:, (ca + (0 if c - ca < 4 else 4)) * 128:(c + 1) * 128],
                                        OTq[:, (0 if c - ca < 4 else 4) * 128:(c - ca + 1) * 128])

                        # qep = OT^T @ qat per chunk (4 per PSUM bank), then
                        # prod = qep * k as one batched DVE op per run
                        prod = prp.tile([128, C * 128], BF16, tag="prod")
                        for (ra, rb) in runs:
                            nb = rb - ra
                            qeq = qeps.tile([128, 512], F32, tag="qeq")
                            for c in range(ra, rb):
                                rc = rels[c]
                                nc.tensor.matmul(
                                    qeq[:, (c - ra) * 128:(c - ra + 1) * 128],
                                    lhsT=ots_sb[:, c * 128:(c + 1) * 128],
                                    rhs=qat[:, rc * D:(rc + 1) * D],
                                    start=True, stop=True,
                                    skip_group_check=True)
                            ka = int(kvg_col[c0 + ra])
                            nc.vector.tensor_tensor(
                                out=prod[:, ra * 128:rb * 128]
                                    .rearrange("p (c x) -> p c x", c=nb),
                                in0=qeq[:, :nb * 128]
                                    .rearrange("p (c x) -> p c x", c=nb),
                                in1=kvgt[:, ka * 256:(ka + nb) * 256]
                                    .rearrange("p (c x) -> p c x", c=nb)[:, :, :128],
                                op=mybir.AluOpType.mult,
                            )

                        attn = smp.tile([128, C * NH], F32, tag="attn")
                        wv = smp.tile([128, C * NH], BF16, tag="wv")
                        for (ca, cb) in runs:
                            nc.vector.reduce_sum(
                                out=attn[:, ca * NH:cb * NH],
                                in_=prod[:, ca * 128:cb * 128]
                                    .rearrange("p (g d) -> p g d", d=DK),
                                axis=mybir.AxisListType.X,
                            )
                            nc.scalar.activation(
                                out=wv[:, ca * NH:cb * NH],
                                in_=attn[:, ca * NH:cb * NH],
                                func=mybir.ActivationFunctionType.Exp)

                        # weighted v, batched per run (alternate DVE/Pool)
                        wmt = wmp.tile([128, C * 128], BF16, tag="wmt")
                        for ri, (ca, cb) in enumerate(runs):
                            nspan = cb - ca
                            ka = int(kvg_col[c0 + ca])
                            eng = nc.vector if ri % 2 == 0 else nc.gpsimd
                            eng.tensor_tensor(
                                out=wmt[:, ca * 128:cb * 128]
                                    .rearrange("p (c h d) -> p c h d", c=nspan, h=NH),
                                in0=kvgt[:, ka * 256:(ka + nspan) * 256]
                                    .rearrange("p (c x) -> p c x", c=nspan)[:, :, 128:256]
                                    .rearrange("p c (h d) -> p c h d", h=NH),
                                in1=wv[:, ca * NH:cb * NH]
                                    .rearrange("p (c h u) -> p c h u", c=nspan, u=1)
                                    .to_broadcast([128, nspan, NH, DK]),
                                op=mybir.AluOpType.mult,
                            )

                        # segment sums: AT[d, j] per relation + s[j, h].
                        # PSUM `start` marks the whole 2KB bank pending-zero,
                        # so each relation's group runs back to back.
                        ATp = atps.tile([128, R * D], F32, tag="ATp")
                        sp = spps.tile([128, 512], F32, tag="sp")
                        by_rel = [[c for c in range(C) if rels[c] == r]
                                  for r in range(R)]
                        for rc in range(R):
                            cs = by_rel[rc]
                            for i, c in enumerate(cs):
                                nc.tensor.matmul(
                                    ATp[:, rc * D:(rc + 1) * D],
                                    lhsT=wmt[:, c * 128:(c + 1) * 128],
                                    rhs=Oall[:, c * 128:(c + 1) * 128],
                                    start=(i == 0), stop=(i == len(cs) - 1),
                                    skip_group_check=True)
                        for c in range(C):
                            nc.tensor.matmul(
                                sp[:, :NH], lhsT=Oall[:, c * 128:(c + 1) * 128],
                                rhs=wv[:, c * NH:(c + 1) * NH],
                                start=(c == 0), stop=(c == C - 1),
                                skip_group_check=True)

                        ssb = smp.tile([128, NH], F32, tag="ssb")
                        nc.vector.tensor_scalar_add(ssb[:], sp[:, :NH], 1e-16)
                        rec = smp.tile([128, NH], F32, tag="rec")
                        nc.vector.reciprocal(rec[:], ssb[:])
                        recx = smp.tile([128, 128], BF16, tag="recx")
                        nc.vector.tensor_copy(
                            recx[:].rearrange("p (h d) -> p h d", h=NH),
                            rec[:].rearrange("p (h u) -> p h u", u=1)
                                .to_broadcast([128, NH, DK]),
                        )
                        rtq = otps.tile([128, 8 * 128], BF16, tag="OTq")
                        nc.tensor.transpose(rtq[:, :128], recx[:], ident[:])
                        rts = smp.tile([128, 128], BF16, tag="rts")
                        nc.scalar.copy(rts[:], rtq[:, :128])

                        Anorm = anp.tile([128, R * D], BF16, tag="Anorm")
                        nc.vector.tensor_tensor(
                            out=Anorm[:].rearrange("p (r j) -> p r j", r=R),
                            in0=ATp[:].rearrange("p (r j) -> p r j", r=R),
                            in1=rts[:].rearrange("p (u j) -> p u j", u=1)
                                .to_broadcast([128, R, 128]),
                            op=mybir.AluOpType.mult,
                        )

                        outp = qps.tile([128, 512], F32, tag="qp")
                        for r in range(R):
                            nc.tensor.matmul(
                                outp[:, :128], lhsT=Anorm[:, r * D:(r + 1) * D],
                                rhs=wmo_sb[:, r * D:(r + 1) * D],
                                start=(r == 0), stop=(r == R - 1))
                        if tl % 2 == 0:
                            osb = smp.tile([128, 256], F32, tag="osb")
                            osb_pair = osb
                        else:
                            osb = osb_pair
                        nc.scalar.copy(osb[:, (tl % 2) * 128:(tl % 2 + 1) * 128],
                                       outp[:, :128])
                        if tl % 2 == 1 or tl == TPC - 1:
                            t0o = tl - (tl % 2)
                            nto = tl - t0o + 1
                            nc.sync.dma_start(
                                out=out_t[:, t0o:t0o + nto, :],
                                in_=osb[:, :nto * 128]
                                    .rearrange("p (t x) -> p t x", t=nto))
            kvgp_cm.__exit__(None, None, None)
    nc.compile()
    return nc


LAST_NC = None
LAST_IN_MAPS = None


def kernel(h, k_linears, q_linears, v_linears, a_linears,
           relation_att, relation_msg, relation_pri, skip,
           row_idx, col_idx, eids, **_unused):
    global LAST_NC, LAST_IN_MAPS
    in_maps, meta = _host_prep(
        h, k_linears, q_linears, v_linears, a_linears,
        relation_att, relation_msg, relation_pri, skip, row_idx, col_idx)
    nc = _build_program(meta)
    LAST_NC, LAST_IN_MAPS = nc, in_maps
    res = run_bass_kernel_spmd(nc, in_maps, list(range(NCORES)))
    out = np.concatenate([res.results[c]["out"] for c in range(NCORES)], axis=0)
    return out.astype(np.float32)
